# revision 26
# baseline (speedup 1.0000x reference)
"""Trainium2 Bass kernel for nn_DBLoss_11605001634022.

DBLoss = Ls + Lb + 10*Lt over four (16,640,640) f32 maps, where Ls/Lb are
"balanced" BCE-with-logits losses with hard-negative mining (keep the top
n_negative = min(n_neg_avail, 3*n_pos) negative losses) and
Lt = mean|thresh - target_thresh|.

For these inputs the targets are ~uniform, so n_neg_avail <= 3*n_pos by a
huge margin and the top-k keeps ALL negatives; each balanced BCE collapses
to a plain mean of the elementwise BCE losses. With
bce(x, t) = softplus(x) - x*t, the whole loss is one streaming reduction:

  loss = [ S(sp(p)) - S(p*tp) + S(sp(50*a)) - 2500*S(a*b) + 10*S(|c|) ] / N
  a = p - t,  b = tp - tt,  c = t - tt,   S = sum over all elements

The kernel verifies the collapse condition on the host (cheap) and falls
back to an exact numpy implementation if it ever fails.

The HW has no softplus ACT table, so softplus uses the relu identity
  S(sp(x)) = (S(x) + S(|x|))/2 + S(ln(1 + exp(-|x|)))
with exp/ln in the single `natural_log_exp_and_others` ACT table set (one
table load, no switches). Likewise
  S(|c|) = 2 S(relu(c)) - S(c) = 2 S(max(t,tt)) - S(tt) - S(t).

Sharded batch-parallel: 2 images/core across 8 cores; each core streams
its 13.1 MB once, in 4 pipelined [128,1600] chunks (5-deep buffered input
tiles, 4-deep intermediates), raw Bass + manual semaphores (the Tile
layer's multi-wait sync is rejected by this walrus). GPSIMD and PE are
kept idle on purpose: GPSIMD elementwise ops measured ~5x slower in situ
than DVE under full SBUF port contention, and the PE column-sum variant
measured slower than host-side float64 input sums. Per-tensor DMA
semaphores let each consumer start as soon as the specific tensor it
needs has landed; all reduce-only outputs write to zero-stride broadcast
dummies to avoid SBUF write-port traffic.
Per chunk:
  DVE  (5 scalar_tensor_tensor ops, each with a free row-sum):
        a=(p*1)-t (+S(a)); (p*-1)*tp (+S); (a*-2500)*tp and (a*2500)*tt
        (+S, the expanded -2500*a*b term); (t*1) max tt (+S(max), for the
        relu identity S(relu(t-tt)) = S(max(t,tt)) - S(tt)).
  ACT  (4 passes): |p| and |50a| (each +row-sum, the latter via the Abs
        pre-scale) into one [128,3200] buffer; one merged exp(-x) pass;
        one merged ln(1+u) pass (+row-sum).
  Host: exact float64 S(t), S(tt) (plain input sums, alongside the
        existing top-k guard scan) close the relu identities.
Row-sums land in per-engine stats tiles (no cross-engine SBUF write
granule sharing), DMA'd out once. Host applies coefficients and the final
division in float64.
"""

import numpy as np

N_CORES = 8
SHAPE = (16, 640, 640)
NTOT = SHAPE[0] * SHAPE[1] * SHAPE[2]
PER_CORE = NTOT // N_CORES  # 819200
P = 128
FDIM = PER_CORE // P  # 6400
NCHUNK = 4
F = FDIM // NCHUNK  # 1600
R = 50.0
ALPHA = 1.0
BETA = 10.0
K = 3

_CACHE = {}


def _get_concourse():
    try:
        import concourse.bass  # noqa: F401
    except ImportError:
        import sys

        sys.path.insert(0, "/opt/trn_rl_repo")
    import concourse.bass as bass
    import concourse.mybir as mybir
    from concourse import bass_utils

    return bass, mybir, bass_utils


def _build(nloop=1):
    """Build the bass program. nloop > 1 repeats the whole pipeline nloop
    times inside one NEFF (same result; used for dispatch-free timing)."""
    if nloop in _CACHE:
        return _CACHE[nloop]
    import contextlib

    bass, mybir, bass_utils = _get_concourse()
    f32 = mybir.dt.float32
    Alu = mybir.AluOpType
    Act = mybir.ActivationFunctionType

    nc = bass.Bass()
    dp = nc.dram_tensor("p", [P, FDIM], f32, kind="ExternalInput")
    dt_ = nc.dram_tensor("t", [P, FDIM], f32, kind="ExternalInput")
    dtp = nc.dram_tensor("tp", [P, FDIM], f32, kind="ExternalInput")
    dtt = nc.dram_tensor("tt", [P, FDIM], f32, kind="ExternalInput")
    dout = nc.dram_tensor("acc_out", [P, 8 * NCHUNK], f32, kind="ExternalOutput")

    NB = 4  # intermediate (tA) buffers
    NBI = 5  # input tile buffers
    T = nloop * NCHUNK

    ctx = contextlib.ExitStack()
    with ctx:
        sb = lambda name, shape: ctx.enter_context(
            nc.sbuf_tensor(name, shape, f32)
        )
        tP = [sb(f"tP{i}", [P, F]) for i in range(NBI)]
        tT = [sb(f"tT{i}", [P, F]) for i in range(NBI)]
        tTP = [sb(f"tTP{i}", [P, F]) for i in range(NBI)]
        tTT = [sb(f"tTT{i}", [P, F]) for i in range(NBI)]
        tA = [sb(f"tA{i}", [P, F]) for i in range(NB)]
        tG = sb("tG", [P, 2 * F])  # [ |p| | |50a| ]
        tE = sb("tE", [P, 2 * F])  # exp outputs (p-half | a-half)
        tF = sb("tF", [P, 1])  # ln dump (broadcast)
        trash = sb("trash", [P, 1])
        acc_d = sb("acc_d", [P, 5 * NCHUNK])
        acc_a = sb("acc_a", [P, 3 * NCHUNK])  # absP, absA, lnC
        dma_p = ctx.enter_context(nc.semaphore())
        dma_t = ctx.enter_context(nc.semaphore())
        dma_tp = ctx.enter_context(nc.semaphore())
        dma_tt = ctx.enter_context(nc.semaphore())
        dve_sem = ctx.enter_context(nc.semaphore())
        act_sem = ctx.enter_context(nc.semaphore())
        block = ctx.enter_context(nc.Block())

        def dcol(j, k):
            return acc_d[:, 5 * j + k : 5 * j + k + 1]

        def acol(j, k):
            return acc_a[:, 3 * j + k : 3 * j + k + 1]

        @block.sync
        def _(sync):
            for jj in range(T):
                j = jj % NCHUNK
                bi = jj % NBI
                sl = slice(j * F, (j + 1) * F)
                if jj >= NBI:
                    # input buffers of chunk jj-NBI must be fully consumed
                    sync.wait_ge(dve_sem, 5 * (jj - 3))
                    sync.wait_ge(act_sem, 4 * (jj - 4) + 1)  # absP read tP
                sync.dma_start(out=tP[bi][:], in_=dp[:, sl]).then_inc(dma_p, 16)
                sync.dma_start(out=tT[bi][:], in_=dt_[:, sl]).then_inc(dma_t, 16)
                sync.dma_start(out=tTP[bi][:], in_=dtp[:, sl]).then_inc(dma_tp, 16)
                sync.dma_start(out=tTT[bi][:], in_=dtt[:, sl]).then_inc(dma_tt, 16)
            sync.wait_ge(dve_sem, 5 * T)
            sync.wait_ge(act_sem, 4 * T)
            sync.dma_start(
                out=dout[:, : 5 * NCHUNK], in_=acc_d[:]
            ).then_inc(dma_p, 16)
            sync.dma_start(
                out=dout[:, 5 * NCHUNK :], in_=acc_a[:]
            ).then_inc(dma_p, 16)
            sync.wait_ge(dma_p, 16 * T + 32)
            sync.wait_ge(dma_t, 16 * T)
            sync.wait_ge(dma_tp, 16 * T)
            sync.wait_ge(dma_tt, 16 * T)

        @block.vector
        def _(vector):
            for jj in range(T):
                j = jj % NCHUNK
                bi = jj % NB
                bii = jj % NBI
                vector.wait_ge(dma_p, 16 * (jj + 1))
                if jj >= NB:
                    # absA of chunk jj-3 must have read tA[bi]
                    vector.wait_ge(act_sem, 4 * (jj - NB) + 2)
                # a = p - t, with free S(a)
                vector.wait_ge(dma_t, 16 * (jj + 1))
                nc.vector.scalar_tensor_tensor(
                    out=tA[bi][:], in0=tP[bii][:], scalar=1.0, in1=tT[bii][:],
                    op0=Alu.mult, op1=Alu.subtract, accum_out=dcol(j, 3),
                ).then_inc(dve_sem, 1)
                # S(-p*tp)
                vector.wait_ge(dma_tp, 16 * (jj + 1))
                nc.vector.scalar_tensor_tensor(
                    out=trash.broadcast_to((P, F)), in0=tP[bii][:], scalar=-1.0, in1=tTP[bii][:],
                    op0=Alu.mult, op1=Alu.mult, accum_out=dcol(j, 0),
                ).then_inc(dve_sem, 1)
                # -2500*S(a*b) expanded: S(-2500*a*tp) + S(2500*a*tt)
                nc.vector.scalar_tensor_tensor(
                    out=trash.broadcast_to((P, F)), in0=tA[bi][:], scalar=-2500.0, in1=tTP[bii][:],
                    op0=Alu.mult, op1=Alu.mult, accum_out=dcol(j, 1),
                ).then_inc(dve_sem, 1)
                vector.wait_ge(dma_tt, 16 * (jj + 1))
                nc.vector.scalar_tensor_tensor(
                    out=trash.broadcast_to((P, F)), in0=tA[bi][:], scalar=2500.0, in1=tTT[bii][:],
                    op0=Alu.mult, op1=Alu.mult, accum_out=dcol(j, 2),
                ).then_inc(dve_sem, 1)
                # S(max(t,tt)): S(relu(t-tt)) = S(max) - S(tt)
                nc.vector.scalar_tensor_tensor(
                    out=trash.broadcast_to((P, F)), in0=tT[bii][:], scalar=1.0, in1=tTT[bii][:],
                    op0=Alu.mult, op1=Alu.max, accum_out=dcol(j, 4),
                ).then_inc(dve_sem, 1)
        @block.scalar
        def _(scalar):
            for jj in range(T):
                j = jj % NCHUNK
                bi = jj % NB
                bii = jj % NBI
                # |p| with free S(|p|)
                scalar.wait_ge(dma_p, 16 * (jj + 1))
                nc.scalar.activation(
                    tG[:, 0:F], tP[bii][:], Act.Abs, accum_out=acol(j, 0)
                ).then_inc(act_sem, 1)
                # |50a| with free S(|50a|)
                scalar.wait_ge(dve_sem, 5 * jj + 1)  # a ready
                nc.scalar.activation(
                    tG[:, F : 2 * F], tA[bi][:], Act.Abs, scale=R,
                    accum_out=acol(j, 1),
                ).then_inc(act_sem, 1)
                # exp(-|p|) | exp(-|50a|) in one pass
                nc.scalar.activation(
                    tE[:], tG[:], Act.Exp, scale=-1.0
                ).then_inc(act_sem, 1)
                nc.scalar.activation(
                    tF.broadcast_to((P, 2 * F)), tE[:], Act.Ln, bias=1.0,
                    accum_out=acol(j, 2),
                ).then_inc(act_sem, 1)

    _CACHE[nloop] = (nc, bass_utils)
    return _CACHE[nloop]


def _run_device(shards, **kwargs):
    """shards: dict name -> list of 8 [P, FDIM] f32 arrays."""
    nc, bass_utils = _build()
    in_maps = [
        {name: shards[name][c] for name in ("p", "t", "tp", "tt")}
        for c in range(N_CORES)
    ]
    return bass_utils.run_bass_kernel_spmd(
        nc, in_maps, core_ids=list(range(N_CORES)), **kwargs
    )


def _shard(arr):
    flat = np.ascontiguousarray(arr, dtype=np.float32).reshape(-1)
    return [
        flat[c * PER_CORE : (c + 1) * PER_CORE].reshape(P, FDIM)
        for c in range(N_CORES)
    ]


def _reduce_host(results, sum_t, sum_tt):
    # acc_out: [0:20] DVE chunk-major (stt1=S(-p*tp), stt2a=S(-2500*a*tp),
    # stt2b=S(2500*a*tt), suma=S(a), smax=S(max(t,tt))), [20:32] ACT
    # chunk-major (absP=S(|p|), abs50A=S(|50a|), lnC=S(ln1p_p)+S(ln1p_a)).
    # sum_t/sum_tt: exact float64 S(t), S(tt) computed on the host.
    #   S(sp(p))   = 0.5 (S(a)+S(t)) + 0.5 S(|p|) + lnC_p
    #   S(sp(50a)) = 25 S(a) + 0.5 S(|50a|) + lnC_a
    #   10 S(|c|)  = 20 S(max(t,tt)) - 10 S(tt) - 10 S(t)
    cd = np.array([1.0, 1.0, 1.0, 0.5 + R / 2.0, 2.0 * BETA])
    ca = np.array([0.5, 0.5, 1.0])  # S(|p|), S(|50a|), lnC
    total = 0.0
    for c in range(N_CORES):
        out = results[c]["acc_out"].astype(np.float64)
        dve = out[:, : 5 * NCHUNK].reshape(P, NCHUNK, 5)
        act = out[:, 5 * NCHUNK :].reshape(P, NCHUNK, 3)
        total += float((dve.sum(axis=(0, 1)) * cd).sum())
        total += float((act.sum(axis=(0, 1)) * ca).sum())
    total += (0.5 - BETA) * sum_t
    total += -BETA * sum_tt
    return np.float32(total / NTOT)


def _numpy_fallback(p, t, tp, tt):
    """Exact reference semantics in float32 numpy (only used if the top-k
    collapse precondition ever fails)."""

    def bce(x, tgt):
        return (
            np.maximum(x, 0.0) - x * tgt + np.log1p(np.exp(-np.abs(x)))
        ).astype(np.float32)

    def balanced(x, tgt):
        losses = bce(x, tgt).ravel()
        mask = tgt.ravel() > 0.5
        n_pos = int(mask.sum())
        n_neg_avail = mask.size - n_pos
        n_negative = min(n_neg_avail, K * n_pos)
        pos_sum = np.float32(losses[mask].sum())
        neg_sorted = np.sort(losses[~mask])[::-1]
        neg_sum = np.float32(neg_sorted[:n_negative].sum())
        return (pos_sum + neg_sum) / np.float32(n_pos + n_negative)

    bin_map = (R * (p - t)).astype(np.float32)
    target_bin = (R * (tp - tt)).astype(np.float32)
    ls = balanced(p, tp)
    lb = balanced(bin_map, target_bin)
    lt = np.abs(t - tt).mean(dtype=np.float32)
    return np.float32(ls + ALPHA * lb + BETA * lt)


def kernel(
    proba_map, thresh_map, target_proba_map, target_thresh_map
) -> np.ndarray:
    p = np.asarray(proba_map, dtype=np.float32)
    t = np.asarray(thresh_map, dtype=np.float32)
    tp = np.asarray(target_proba_map, dtype=np.float32)
    tt = np.asarray(target_thresh_map, dtype=np.float32)

    # The device kernel assumes the hard-negative top-k keeps every negative
    # (n_neg_avail <= K*n_pos for both BCE terms). Cheap host check; exact
    # fallback otherwise.
    npos1 = int(np.count_nonzero(tp > 0.5))
    d = (R * (tp - tt)).astype(np.float32)
    npos2 = int(np.count_nonzero(d > 0.5))
    if (tp.size - npos1) > K * npos1 or (d.size - npos2) > K * npos2:
        return _numpy_fallback(p, t, tp, tt)

    sum_t = float(np.sum(t, dtype=np.float64))
    sum_tt = float(np.sum(tt, dtype=np.float64))
    shards = {"p": _shard(p), "t": _shard(t), "tp": _shard(tp), "tt": _shard(tt)}
    res = _run_device(shards)
    return _reduce_host(res.results, sum_t, sum_tt)
